# revision 25
# baseline (speedup 1.0000x reference)
"""DualAttention Trainium2 kernel (8 NeuronCores, data-parallel over batch).

Math (per batch b, head h, dk=64, S=1024):
  s   = (q @ k^T) / 8                      [S, S]
  E   = exp(s) with strict-causal mask (j < i) applied as -1e30 pre-exp
  Z1  = rowsum(E)                          (row 0: Z1=0 -> handled specially)
  p1  = (E / Z1) * notcm                   (notcm zeroes counter-masked key cols)
  E2  = exp(p1)  -- dense: exp(0)=1 for all masked/future cols
  Z2  = sum_j E2[j] = rowsum_window(E2) + (S - W)
  out = (E2 @ v)/Z2 = (E2|window @ v|window + colsum_{j>=W} v)/Z2
  row 0 of out is forced to 0 (reference zeroes p row 0 post-softmax).

Kernel strategy per core (1 batch item): loop 8 head-pairs; per head:
scores via PE (bf16), causal -1e30 via a const matmul addend, exp1 on ACT
with fused accum -> Z1, counter-mask+1/Z1 fused in one DVE
scalar_tensor_tensor, one batched exp2 on ACT, DMA-xbar transposes of E2
(bf16) for the P@V matmuls, rank-8 suffix-colsum correction as a K=8
matmul, Z2 via a ones-column matmul, final 1/Z2 on DVE.
"""

import numpy as np

import concourse.bass as bass
import concourse.mybir as mybir
from concourse.tile import TileContext
from concourse.alu_op_type import AluOpType

F32 = mybir.dt.float32
BF16 = mybir.dt.bfloat16

B, S, D = 8, 1024, 1024
H, DK = 16, 64
NCORES = 8
P = 128          # partition block
NQB = S // P     # 8 query blocks
MASKADD = -1e30
# packed offsets for the causal windows W=(qb+1)*128
OFF = [0]
for _qb in range(NQB):
    OFF.append(OFF[-1] + (_qb + 1) * P)
TOTW = OFF[-1]   # 4608


def build_nc():
    from concourse.bacc import Bacc

    nc = Bacc()
    # host passes q/k pre-transposed [D, S] and everything pre-cast to bf16
    qt_d = nc.declare_dram_parameter("qT", [D, S], BF16, isOutput=False)
    kt_d = nc.declare_dram_parameter("kT", [D, S], BF16, isOutput=False)
    v1_d = nc.declare_dram_parameter("v1", [S, D], BF16, isOutput=False)
    v2_d = nc.declare_dram_parameter("v2", [S, D], BF16, isOutput=False)
    cm_d = nc.declare_dram_parameter("cm", [1, S], F32, isOutput=False)
    o1_d = nc.declare_dram_parameter("out1", [S, D], F32, isOutput=True)
    o2_d = nc.declare_dram_parameter("out2", [S, D], F32, isOutput=True)

    from contextlib import ExitStack

    with TileContext(nc) as tc, ExitStack() as ctx:
        const = ctx.enter_context(tc.tile_pool(name="const", bufs=1))
        hpool = ctx.enter_context(tc.tile_pool(name="hp", bufs=2))
        epool = ctx.enter_context(tc.tile_pool(name="ep", bufs=10))
        packp = ctx.enter_context(tc.tile_pool(name="pk", bufs=2))
        etp = ctx.enter_context(tc.tile_pool(name="et", bufs=2))
        smol = ctx.enter_context(tc.tile_pool(name="sm", bufs=4))
        outp = ctx.enter_context(tc.tile_pool(name="op", bufs=3))
        ps_pool = ctx.enter_context(tc.tile_pool(name="ps", bufs=2, space="PSUM"))
        po_pool = ctx.enter_context(tc.tile_pool(name="po", bufs=1, space="PSUM"))
        pz_pool = ctx.enter_context(tc.tile_pool(name="pz", bufs=1, space="PSUM"))
        pc_pool = ctx.enter_context(tc.tile_pool(name="pc", bufs=1, space="PSUM"))

        # ---------------- constants ----------------
        ident = const.tile([P, P], BF16, tag="ident")
        nc.gpsimd.memset(ident[:], 0.0)
        nc.gpsimd.affine_select(
            out=ident[:], in_=ident[:], compare_op=AluOpType.not_equal,
            fill=1.0, base=0, pattern=[[-1, P]], channel_multiplier=1)

        # tric[r, c] = -1e30 where c >= r (strict causal: only j < i survives)
        # keep 0 where r - c - 1 >= 0 (c < r), else fill -1e30 (c >= r)
        tric = const.tile([P, P], BF16, tag="tric")
        nc.gpsimd.memset(tric[:], 0.0)
        nc.gpsimd.affine_select(
            out=tric[:], in_=tric[:], compare_op=AluOpType.is_ge,
            fill=MASKADD, base=-1, pattern=[[-1, P]], channel_multiplier=1)

        # onehot: 8 blocks [128, 8]; block c has column c all-ones
        onehot = const.tile([P, 64], BF16, tag="onehot")
        nc.gpsimd.memset(onehot[:], 0.0)
        for c in range(NQB):
            nc.gpsimd.memset(onehot[:, c * 8 + c : c * 8 + c + 1], 1.0)

        # stairs[c, qb*128 + j] = 1 where c > qb  (suffix-sum selector)
        # condition c > floor(x/128)  <=>  128*c - x - 1 >= 0
        stairs = const.tile([NQB, S], BF16, tag="stairs")
        nc.gpsimd.memset(stairs[:], 1.0)
        nc.gpsimd.affine_select(
            out=stairs[:], in_=stairs[:], compare_op=AluOpType.is_ge,
            fill=0.0, base=-1, pattern=[[-1, S]], channel_multiplier=P)

        ones_col = const.tile([P, 1], BF16, tag="onescol")
        nc.gpsimd.memset(ones_col[:], 1.0)

        # wconst[:, qb] = S - (qb+1)*128  (the "+(S-W)" part of Z2)
        wconst = const.tile([P, NQB], F32, tag="wconst")
        for qb in range(NQB):
            nc.gpsimd.memset(wconst[:, qb : qb + 1], float(S - (qb + 1) * P))

        # ------------- counter-mask broadcast [128, S] (bf16) -------------
        cmrow = const.tile([1, S], F32, tag="cmrow")
        nc.sync.dma_start(out=cmrow[:], in_=cm_d[:])
        cmrow16 = const.tile([1, S], BF16, tag="cmrow16")
        nc.gpsimd.tensor_copy(cmrow16[:], cmrow[:])
        ones_row16 = const.tile([1, P], BF16, tag="onesrow16")
        nc.gpsimd.memset(ones_row16[:], 1.0)
        cmb = const.tile([P, S], BF16, tag="cmb")
        ps_cm = ps_pool.tile([P, S], F32, tag="ps")
        for half in range(2):
            sl = slice(half * 512, (half + 1) * 512)
            nc.tensor.matmul(ps_cm[:, sl], ones_row16[:], cmrow16[:, sl],
                             start=True, stop=True)
        nc.vector.tensor_copy(cmb[:], ps_cm[:])

        # ---------------- main loop: 8 head-pairs ----------------
        for hp in range(NQB):
            dsl = slice(hp * P, (hp + 1) * P)  # d-columns of this head pair

            # qT2[d_local, s]: one contiguous DMA; d_local = this head pair
            qT2 = hpool.tile([P, S], BF16, tag="qT2")
            kT2 = hpool.tile([P, S], BF16, tag="kT2")
            nc.sync.dma_start(out=qT2[:], in_=qt_d[dsl, :])
            nc.sync.dma_start(out=kT2[:], in_=kt_d[dsl, :])
            # v tiles: SBUF[p, (c,d)] = DRAM[c*128+p, d] in one DMA each
            v1b = hpool.tile([P, S], BF16, tag="v1b")
            v2b = hpool.tile([P, S], BF16, tag="v2b")
            for t_sb, t_dr in ((v1b, v1_d), (v2b, v2_d)):
                nc.sync.dma_start(
                    out=t_sb.rearrange("p (c d) -> p c d", c=NQB),
                    in_=t_dr[:, dsl].rearrange("(c s) d -> s c d", c=NQB))

            for hl in range(2):  # the two heads of the pair
                h = 2 * hp + hl
                pb = hl * DK  # partition base of this head's d rows in qT2/kT2

                # --- chunk colsums of [v1|v2] -> cs_sb [8, 128] (bf16) ---
                pcs = pc_pool.tile([NQB, P], F32, tag="pc")
                # one start marks the whole psum bank pending-zero: first
                # touch of each byte overwrites, later touches accumulate
                for c in range(NQB):
                    lhs = onehot[:, c * 8 : (c + 1) * 8]
                    va16 = v1b[:, c * P + pb : c * P + pb + DK]
                    vb16 = v2b[:, c * P + pb : c * P + pb + DK]
                    nc.tensor.matmul(pcs[:, 0:DK], lhs, va16,
                                     start=(c == 0), stop=False)
                    nc.tensor.matmul(pcs[:, DK:P], lhs, vb16,
                                     start=False, stop=(c == NQB - 1))
                cs_sb = smol.tile([NQB, P], BF16, tag="cs")
                nc.vector.tensor_copy(cs_sb[:], pcs[:])

                z1 = smol.tile([P, NQB], F32, tag="z1")
                r1 = smol.tile([P, NQB], F32, tag="r1")
                etiles = []

                # --- P1: scores + causal add + exp1(accum->Z1) per qb ---
                for qb in range(NQB):
                    W = (qb + 1) * P
                    ps = ps_pool.tile([P, S], F32, tag="ps")
                    # split at the 512-col psum bank boundary
                    for lo in range(0, W, 512):
                        hi = min(lo + 512, W)
                        nc.tensor.matmul(
                            ps[:, lo:hi],
                            qT2[pb : pb + DK, qb * P : (qb + 1) * P],
                            kT2[pb : pb + DK, lo:hi],
                            start=True, stop=(hi < W))
                    # add -1e30 to j >= i inside the diagonal block
                    nc.tensor.matmul(
                        ps[:, W - P : W], ident[:], tric[:],
                        start=False, stop=True)
                    e_t = epool.tile([P, S], BF16, tag="E")
                    nc.scalar.activation(
                        out=e_t[:, 0:W], in_=ps[:, 0:W],
                        func=mybir.ActivationFunctionType.Exp,
                        scale=0.125, accum_out=z1[:, qb : qb + 1])
                    etiles.append(e_t)

                # --- P2: R1 = 1/Z1 ; row0 of qb0 gets scale 0 ---
                nc.vector.reciprocal(r1[:], z1[:])
                nc.gpsimd.memset(r1[0:1, 0:1], 0.0)

                # --- P3: fused (E * 1/Z1) * notcm -> packed Ecm ---
                ecm = packp.tile([P, TOTW], BF16, tag="ecm")
                for qb in range(NQB):
                    W = (qb + 1) * P
                    nc.vector.scalar_tensor_tensor(
                        out=ecm[:, OFF[qb] : OFF[qb] + W],
                        in0=etiles[qb][:, 0:W],
                        scalar=r1[:, qb : qb + 1],
                        in1=cmb[:, 0:W],
                        op0=AluOpType.mult, op1=AluOpType.mult)

                # --- P4: one batched exp2 ---
                e2 = packp.tile([P, TOTW], BF16, tag="e2")
                nc.scalar.activation(
                    out=e2[:], in_=ecm[:],
                    func=mybir.ActivationFunctionType.Exp)

                # --- P5/P6/P7: transpose E2, P@V, Z2 col, suffix corr ---
                po = po_pool.tile([P, S], F32, tag="po")
                pz = pz_pool.tile([P, NQB], F32, tag="pz")
                for qb in range(NQB):
                    W = (qb + 1) * P
                    e2t = etp.tile([P, S], BF16, tag="e2t")
                    for kc in range(qb + 1):
                        nc.sync.dma_start(
                            out=e2t[:, kc * P : (kc + 1) * P],
                            in_=e2[:, OFF[qb] + kc * P : OFF[qb] + (kc + 1) * P],
                            transpose=True)
                    # po is 2 psum banks (qb 0-3 / 4-7): start only on the
                    # first touch of each bank, stop only on its last touch
                    for kc in range(qb + 1):
                        lhs = e2t[:, kc * P : (kc + 1) * P]
                        va = v1b[:, kc * P + pb : kc * P + pb + DK]
                        vb = v2b[:, kc * P + pb : kc * P + pb + DK]
                        first_bank = kc == 0 and qb % 4 == 0
                        last_bank = qb == NQB - 1 and kc == qb
                        nc.tensor.matmul(po[:, qb * P : qb * P + DK], lhs, va,
                                         start=first_bank, stop=False)
                        nc.tensor.matmul(po[:, qb * P + DK : (qb + 1) * P],
                                         lhs, vb, start=False, stop=last_bank)
                        nc.tensor.matmul(pz[:, qb : qb + 1], lhs, ones_col[:],
                                         start=(qb == 0 and kc == 0),
                                         stop=(qb == NQB - 1 and kc == qb))
                    if qb < NQB - 1:
                        # += sum_{keys >= W} v  (rank-8 via stairs selector)
                        nc.tensor.matmul(
                            po[:, qb * P : (qb + 1) * P],
                            stairs[:, qb * P : (qb + 1) * P], cs_sb[:],
                            start=False, stop=(qb == 3))

                # --- P8: Z2 = pz + (S - W); R2 = 1/Z2 ---
                z2 = smol.tile([P, NQB], F32, tag="z2")
                r2 = smol.tile([P, NQB], F32, tag="r2")
                nc.vector.tensor_tensor(
                    out=z2[:], in0=pz[:], in1=wconst[:], op=AluOpType.add)
                nc.vector.reciprocal(r2[:], z2[:])

                # --- P9: scale by 1/Z2, zero row 0, store ---
                for qb in range(NQB):
                    osb = outp.tile([P, P], F32, tag="osb")
                    nc.vector.tensor_scalar_mul(
                        osb[:], po[:, qb * P : (qb + 1) * P],
                        r2[:, qb : qb + 1])
                    if qb == 0:
                        nc.gpsimd.memset(osb[0:1, :], 0.0)
                    rows = slice(qb * P, (qb + 1) * P)
                    cols = slice(h * DK, (h + 1) * DK)
                    nc.sync.dma_start(out=o1_d[rows, cols], in_=osb[:, 0:DK])
                    nc.sync.dma_start(out=o2_d[rows, cols], in_=osb[:, DK:P])
    nc.compile()
    return nc


_NC_CACHE = None


def _get_nc():
    global _NC_CACHE
    if _NC_CACHE is None:
        _NC_CACHE = build_nc()
    return _NC_CACHE


def prep_inputs(q, k, v1, v2, counter_attention_mask):
    """Host-side shard prep: transpose q/k per batch, cast all to bf16."""
    import ml_dtypes

    bf = ml_dtypes.bfloat16
    q = np.asarray(q, dtype=np.float32)
    k = np.asarray(k, dtype=np.float32)
    v1 = np.asarray(v1, dtype=np.float32).astype(bf)
    v2 = np.asarray(v2, dtype=np.float32).astype(bf)
    cm = np.asarray(counter_attention_mask)
    notcm = (cm == 0).astype(np.float32)  # [B, S]
    return [
        {"qT": np.ascontiguousarray(q[b].astype(bf).T),
         "kT": np.ascontiguousarray(k[b].astype(bf).T),
         "v1": v1[b], "v2": v2[b],
         "cm": notcm[b : b + 1, :]}
        for b in range(NCORES)
    ]


def kernel(q, k, v1, v2, counter_attention_mask):
    from concourse.bass_utils import run_bass_kernel_spmd

    in_maps = prep_inputs(q, k, v1, v2, counter_attention_mask)
    nc = _get_nc()
    res = run_bass_kernel_spmd(nc, in_maps, list(range(NCORES))).results
    out1 = np.stack([res[b]["out1"] for b in range(NCORES)])
    out2 = np.stack([res[b]["out2"] for b in range(NCORES)])
    return out1, out2


# revision 37
# speedup vs baseline: 1.0208x; 1.0208x over previous
"""DualAttention Trainium2 kernel (8 NeuronCores, data-parallel over batch).

Math (per batch b, head h, dk=64, S=1024):
  s   = (q @ k^T) / 8                      [S, S]
  E   = exp(s) with strict-causal mask (j < i) applied as -1e30 pre-exp
  Z1  = rowsum(E)                          (row 0: Z1=0 -> handled specially)
  p1  = (E / Z1) * notcm                   (notcm zeroes counter-masked key cols)
  E2  = exp(p1)  -- dense: exp(0)=1 for all masked/future cols
  Z2  = sum_j E2[j] = rowsum_window(E2) + (S - W)
  out = (E2 @ v)/Z2 = (E2|window @ v|window + colsum_{j>=W} v)/Z2
  row 0 of out is forced to 0 (reference zeroes p row 0 post-softmax).

Kernel strategy per core (1 batch item): loop 8 head-pairs; per head:
scores via PE (bf16), causal -1e30 via a const matmul addend, exp1 on ACT
with fused accum -> Z1, counter-mask+1/Z1 fused in one DVE
scalar_tensor_tensor, one batched exp2 on ACT, DMA-xbar transposes of E2
(bf16) for the P@V matmuls, rank-8 suffix-colsum correction as a K=8
matmul, Z2 via a ones-column matmul, final 1/Z2 on DVE.
"""

import numpy as np

import concourse.bass as bass
import concourse.mybir as mybir
from concourse.tile import TileContext
from concourse.alu_op_type import AluOpType

F32 = mybir.dt.float32
BF16 = mybir.dt.bfloat16

B, S, D = 8, 1024, 1024
H, DK = 16, 64
NCORES = 8
P = 128          # partition block
NQB = S // P     # 8 query blocks
MASKADD = -1e30
# packed offsets for the causal windows W=(qb+1)*128
OFF = [0]
for _qb in range(NQB):
    OFF.append(OFF[-1] + (_qb + 1) * P)
TOTW = OFF[-1]   # 4608


def build_nc():
    from concourse.bacc import Bacc

    nc = Bacc()
    # host passes q/k pre-transposed [D, S] and everything pre-cast to bf16
    qt_d = nc.declare_dram_parameter("qT", [D, S], BF16, isOutput=False)
    kt_d = nc.declare_dram_parameter("kT", [D, S], BF16, isOutput=False)
    v1_d = nc.declare_dram_parameter("v1", [S, D], BF16, isOutput=False)
    v2_d = nc.declare_dram_parameter("v2", [S, D], BF16, isOutput=False)
    cm_d = nc.declare_dram_parameter("cm", [1, S], F32, isOutput=False)
    o1_d = nc.declare_dram_parameter("out1", [S, D], F32, isOutput=True)
    o2_d = nc.declare_dram_parameter("out2", [S, D], F32, isOutput=True)

    from contextlib import ExitStack

    with TileContext(nc) as tc, ExitStack() as ctx:
        const = ctx.enter_context(tc.tile_pool(name="const", bufs=1))
        qkpool = ctx.enter_context(tc.tile_pool(name="qk", bufs=2))
        hpool = ctx.enter_context(tc.tile_pool(name="hp", bufs=3))
        epool = ctx.enter_context(tc.tile_pool(name="ep", bufs=16))
        packp = ctx.enter_context(tc.tile_pool(name="pk", bufs=2))
        etp = ctx.enter_context(tc.tile_pool(name="et", bufs=2))
        smol = ctx.enter_context(tc.tile_pool(name="sm", bufs=6))
        outp = ctx.enter_context(tc.tile_pool(name="op", bufs=2))
        bigp = ctx.enter_context(tc.tile_pool(name="big", bufs=1))
        # PSUM budget (8 banks): ps 2x2 + po 2 + small 2x1
        ps_pool = ctx.enter_context(tc.tile_pool(name="ps", bufs=2, space="PSUM"))
        po_pool = ctx.enter_context(tc.tile_pool(name="po", bufs=1, space="PSUM"))
        pc_pool = ctx.enter_context(tc.tile_pool(name="pc", bufs=2, space="PSUM"))

        # ---------------- constants ----------------
        # touch Exp immediately so the ~2.7us ACT table load overlaps the
        # first input DMAs instead of stalling the first exp1
        warm = const.tile([1, 1], F32, tag="warm")
        nc.gpsimd.memset(warm[:], 0.0)
        nc.scalar.activation(out=warm[:], in_=warm[:],
                             func=mybir.ActivationFunctionType.Exp)

        ident = const.tile([P, P], BF16, tag="ident")
        nc.gpsimd.memset(ident[:], 0.0)
        nc.gpsimd.affine_select(
            out=ident[:], in_=ident[:], compare_op=AluOpType.not_equal,
            fill=1.0, base=0, pattern=[[-1, P]], channel_multiplier=1)

        # tric[r, c] = -1e30 where c >= r (strict causal: only j < i survives)
        # keep 0 where r - c - 1 >= 0 (c < r), else fill -1e30 (c >= r)
        tric = const.tile([P, P], BF16, tag="tric")
        nc.gpsimd.memset(tric[:], 0.0)
        nc.gpsimd.affine_select(
            out=tric[:], in_=tric[:], compare_op=AluOpType.is_ge,
            fill=MASKADD, base=-1, pattern=[[-1, P]], channel_multiplier=1)

        # onehot: 8 blocks [128, 8]; block c has column c all-ones
        onehot = const.tile([P, 64], BF16, tag="onehot")
        nc.gpsimd.memset(onehot[:], 0.0)
        for c in range(NQB):
            nc.gpsimd.memset(onehot[:, c * 8 + c : c * 8 + c + 1], 1.0)

        # stairs[c, qb*128 + j] = 1 where c > qb  (suffix-sum selector)
        # condition c > floor(x/128)  <=>  128*c - x - 1 >= 0
        stairs = const.tile([NQB, S], BF16, tag="stairs")
        nc.gpsimd.memset(stairs[:], 1.0)
        nc.gpsimd.affine_select(
            out=stairs[:], in_=stairs[:], compare_op=AluOpType.is_ge,
            fill=0.0, base=-1, pattern=[[-1, S]], channel_multiplier=P)

        ones_col = const.tile([P, 1], BF16, tag="onescol")
        nc.gpsimd.memset(ones_col[:], 1.0)

        # wconst[:, qb] = S - (qb+1)*128  (the "+(S-W)" part of Z2)
        wconst = const.tile([P, NQB], F32, tag="wconst")
        for qb in range(NQB):
            nc.gpsimd.memset(wconst[:, qb : qb + 1], float(S - (qb + 1) * P))

        # ------------- counter-mask broadcast [128, S] (bf16) -------------
        cmrow = const.tile([1, S], F32, tag="cmrow")
        nc.sync.dma_start(out=cmrow[:], in_=cm_d[:])
        cmrow16 = const.tile([1, S], BF16, tag="cmrow16")
        nc.gpsimd.tensor_copy(cmrow16[:], cmrow[:])
        ones_row16 = const.tile([1, P], BF16, tag="onesrow16")
        nc.gpsimd.memset(ones_row16[:], 1.0)
        cmb = const.tile([P, S], BF16, tag="cmb")
        ps_cm = ps_pool.tile([P, S], F32, tag="ps")
        for half in range(2):
            sl = slice(half * 512, (half + 1) * 512)
            nc.tensor.matmul(ps_cm[:, sl], ones_row16[:], cmrow16[:, sl],
                             start=True, stop=True)
        nc.vector.tensor_copy(cmb[:], ps_cm[:])

        # ------------- main loop: 16 heads, 3-stage software pipeline ------
        # A(h): scores + causal + exp1 (+loads, colsums). B(h): 1/Z1, cmmul,
        # exp2, transpose, P@V. C(h): 1/Z2, scale, store. Emitting
        # A(h), C(h-2), B(h-1) keeps each engine's FIFO free of stalls.
        state = {}
        # full outputs accumulate in SBUF; flushed in 1KB-run DMAs per
        # 4-head group (strided 256B-row writes are ~4x slower)
        big1 = bigp.tile([P, NQB * S], F32, tag="big1")
        big2 = bigp.tile([P, NQB * S], F32, tag="big2")

        def stage_a(h):
            hp, hl = divmod(h, 2)
            if hl == 0:
                dsl = slice(hp * P, (hp + 1) * P)
                qT2 = qkpool.tile([P, S], BF16, tag="qT2")
                kT2 = qkpool.tile([P, S], BF16, tag="kT2")
                nc.sync.dma_start(out=qT2[:], in_=qt_d[dsl, :])
                nc.sync.dma_start(out=kT2[:], in_=kt_d[dsl, :])
                # v tiles: SBUF[p, (c,d)] = DRAM[c*128+p, d], one DMA each
                v1b = hpool.tile([P, S], BF16, tag="v1b")
                v2b = hpool.tile([P, S], BF16, tag="v2b")
                for t_sb, t_dr in ((v1b, v1_d), (v2b, v2_d)):
                    nc.sync.dma_start(
                        out=t_sb.rearrange("p (c d) -> p c d", c=NQB),
                        in_=t_dr[:, dsl].rearrange("(c s) d -> s c d", c=NQB))
                state[("pair", hp)] = (qT2, kT2, v1b, v2b)
            qT2, kT2, v1b, v2b = state[("pair", hp)]
            pb = hl * DK  # partition base of this head inside the pair

            # chunk colsums of [v1|v2] -> cs_sb [8, 128] (bf16). One start
            # marks the whole psum bank pending-zero: first touch of each
            # byte overwrites, later touches accumulate.
            pcs = pc_pool.tile([P, P], F32, tag="small")
            for c in range(NQB):
                lhs = onehot[:, c * 8 : (c + 1) * 8]
                va16 = v1b[:, c * P + pb : c * P + pb + DK]
                vb16 = v2b[:, c * P + pb : c * P + pb + DK]
                nc.tensor.matmul(pcs[0:NQB, 0:DK], lhs, va16,
                                 start=(c == 0), stop=False)
                nc.tensor.matmul(pcs[0:NQB, DK:P], lhs, vb16,
                                 start=False, stop=(c == NQB - 1))
            cs_sb = smol.tile([NQB, P], BF16, tag="cs")
            nc.vector.tensor_copy(cs_sb[:], pcs[0:NQB, :])

            z1 = smol.tile([P, NQB], F32, tag="z1")
            etiles = []
            for qb in range(NQB):
                W = (qb + 1) * P
                ps = ps_pool.tile([P, S], F32, tag="ps")
                # split at the 512-col psum bank boundary
                for lo in range(0, W, 512):
                    hi = min(lo + 512, W)
                    nc.tensor.matmul(
                        ps[:, lo:hi],
                        qT2[pb : pb + DK, qb * P : (qb + 1) * P],
                        kT2[pb : pb + DK, lo:hi],
                        start=True, stop=(hi < W))
                # add -1e30 to j >= i inside the diagonal block
                nc.tensor.matmul(
                    ps[:, W - P : W], ident[:], tric[:],
                    start=False, stop=True)
                e_t = epool.tile([P, S], BF16, tag="E")
                nc.scalar.activation(
                    out=e_t[:, 0:W], in_=ps[:, 0:W],
                    func=mybir.ActivationFunctionType.Exp,
                    scale=0.125, accum_out=z1[:, qb : qb + 1])
                etiles.append(e_t)
            state[h] = dict(pb=pb, v1b=v1b, v2b=v2b, cs_sb=cs_sb,
                            z1=z1, etiles=etiles)

        def stage_b(h):
            st = state[h]
            pb, v1b, v2b = st["pb"], st["v1b"], st["v2b"]

            r1 = smol.tile([P, NQB], F32, tag="r1")
            nc.vector.reciprocal(r1[:], st["z1"][:])
            if True:  # query row 0 has Z1=0; force scale 0 (out row zeroed)
                nc.gpsimd.memset(r1[0:1, 0:1], 0.0)

            # fused (E * 1/Z1) * notcm -> packed pp; exp2 runs in place
            pp = packp.tile([P, TOTW], BF16, tag="pp")
            for qb in range(NQB):
                W = (qb + 1) * P
                nc.vector.scalar_tensor_tensor(
                    out=pp[:, OFF[qb] : OFF[qb] + W],
                    in0=st["etiles"][qb][:, 0:W],
                    scalar=r1[:, qb : qb + 1],
                    in1=cmb[:, 0:W],
                    op0=AluOpType.mult, op1=AluOpType.mult)

            # exp2 in place, split so early query blocks unblock transposes
            nc.scalar.activation(out=pp[:, 0 : OFF[4]], in_=pp[:, 0 : OFF[4]],
                                 func=mybir.ActivationFunctionType.Exp)
            nc.scalar.activation(out=pp[:, OFF[4] :], in_=pp[:, OFF[4] :],
                                 func=mybir.ActivationFunctionType.Exp)

            # all 36 (qb, kc) chunks transposed in two blocked DMAs
            e2t = etp.tile([P, TOTW], BF16, tag="e2t")
            NB4 = OFF[4] // P  # 10 chunks in qb 0..3
            nc.sync.dma_start(
                out=e2t[:, 0 : OFF[4]].rearrange("p (n s) -> p n s", n=NB4),
                in_=pp[:, 0 : OFF[4]].rearrange("p (n s) -> p n s", n=NB4),
                transpose=True)
            nc.sync.dma_start(
                out=e2t[:, OFF[4] :].rearrange("p (n s) -> p n s",
                                               n=TOTW // P - NB4),
                in_=pp[:, OFF[4] :].rearrange("p (n s) -> p n s",
                                              n=TOTW // P - NB4),
                transpose=True)

            # P@[v1|v2] + Z2 ones-column + suffix correction
            po = po_pool.tile([P, S], F32, tag="po")
            pz = pc_pool.tile([P, P], F32, tag="small")
            for qb in range(NQB):
                for kc in range(qb + 1):
                    n = OFF[qb] // P + kc
                    lhs = e2t[:, n * P : (n + 1) * P]
                    va = v1b[:, kc * P + pb : kc * P + pb + DK]
                    vb = v2b[:, kc * P + pb : kc * P + pb + DK]
                    first_bank = kc == 0 and qb % 4 == 0
                    last_bank = qb == NQB - 1 and kc == qb
                    nc.tensor.matmul(po[:, qb * P : qb * P + DK], lhs, va,
                                     start=first_bank, stop=False)
                    nc.tensor.matmul(po[:, qb * P + DK : (qb + 1) * P],
                                     lhs, vb, start=False, stop=last_bank)
                    nc.tensor.matmul(pz[:, qb : qb + 1], lhs, ones_col[:],
                                     start=(qb == 0 and kc == 0),
                                     stop=(qb == NQB - 1 and kc == qb))
                if qb < NQB - 1:
                    # += sum_{keys >= W} v  (rank-8 via stairs selector)
                    nc.tensor.matmul(
                        po[:, qb * P : (qb + 1) * P],
                        stairs[:, qb * P : (qb + 1) * P], st["cs_sb"][:],
                        start=False, stop=(qb == 3))
            st.update(po=po, pz=pz)

        def stage_c(h):
            st = state.pop(h)
            po, pz = st["po"], st["pz"]
            z2 = smol.tile([P, NQB], F32, tag="z2")
            r2 = smol.tile([P, NQB], F32, tag="r2")
            nc.vector.tensor_tensor(
                out=z2[:], in0=pz[0:P, 0:NQB], in1=wconst[:],
                op=AluOpType.add)
            nc.vector.reciprocal(r2[:], z2[:])

            obuf = outp.tile([P, S], F32, tag="osb")
            for qb in range(NQB):
                nc.vector.tensor_scalar_mul(
                    obuf[:, qb * P : (qb + 1) * P],
                    po[:, qb * P : (qb + 1) * P],
                    r2[:, qb : qb + 1])
            # spread into the big output accumulators (gpsimd is idle)
            ob3 = obuf.rearrange("p (c x) -> p c x", c=NQB)
            b13 = big1.rearrange("p (c d) -> p c d", c=NQB)
            b23 = big2.rearrange("p (c d) -> p c d", c=NQB)
            hc = slice(h * DK, (h + 1) * DK)
            nc.gpsimd.tensor_copy(b13[:, :, hc], ob3[:, :, 0:DK])
            nc.gpsimd.tensor_copy(b23[:, :, hc], ob3[:, :, DK:P])
            nc.gpsimd.memset(big1[0:1, h * DK : (h + 1) * DK], 0.0)
            nc.gpsimd.memset(big2[0:1, h * DK : (h + 1) * DK], 0.0)
            if h % 2 == 1:
                g = slice((h - 1) * DK, (h + 1) * DK)
                nc.sync.dma_start(
                    out=o1_d[:, g].rearrange("(c s) d -> s c d", c=NQB),
                    in_=b13[:, :, g])
                nc.sync.dma_start(
                    out=o2_d[:, g].rearrange("(c s) d -> s c d", c=NQB),
                    in_=b23[:, :, g])

        for it in range(H + 2):
            if it < H:
                stage_a(it)
            if it >= 2:
                stage_c(it - 2)
            if 1 <= it <= H:
                stage_b(it - 1)
    nc.compile()
    return nc


_NC_CACHE = None


def _get_nc():
    global _NC_CACHE
    if _NC_CACHE is None:
        _NC_CACHE = build_nc()
    return _NC_CACHE


def prep_inputs(q, k, v1, v2, counter_attention_mask):
    """Host-side shard prep: transpose q/k per batch, cast all to bf16."""
    import ml_dtypes

    bf = ml_dtypes.bfloat16
    q = np.asarray(q, dtype=np.float32)
    k = np.asarray(k, dtype=np.float32)
    v1 = np.asarray(v1, dtype=np.float32).astype(bf)
    v2 = np.asarray(v2, dtype=np.float32).astype(bf)
    cm = np.asarray(counter_attention_mask)
    notcm = (cm == 0).astype(np.float32)  # [B, S]
    return [
        {"qT": np.ascontiguousarray(q[b].astype(bf).T),
         "kT": np.ascontiguousarray(k[b].astype(bf).T),
         "v1": v1[b], "v2": v2[b],
         "cm": notcm[b : b + 1, :]}
        for b in range(NCORES)
    ]


def kernel(q, k, v1, v2, counter_attention_mask):
    from concourse.bass_utils import run_bass_kernel_spmd

    in_maps = prep_inputs(q, k, v1, v2, counter_attention_mask)
    nc = _get_nc()
    res = run_bass_kernel_spmd(nc, in_maps, list(range(NCORES))).results
    out1 = np.stack([res[b]["out1"] for b in range(NCORES)])
    out2 = np.stack([res[b]["out2"] for b in range(NCORES)])
    return out1, out2
